# revision 8
# baseline (speedup 1.0000x reference)
"""CapsuleLayer dynamic-routing kernel for 8 TRN2 NeuronCores — zero-collective.

Under this stack's cost model a collective costs 15us (+1.875x for AllReduce)
regardless of payload, while a full-problem u_hat-sized matmul streams in
~9.6us.  So instead of sharding the routing (3 collectives on the critical
path), every core redundantly computes routing rounds 1-2 on the FULL batch
and full in_size, and only round 3 — whose output v3 is the kernel result and
has no cross-core dependency — is sharded, over out_size d (2 of 16 per core).

The d-shard costs nothing extra: all contractions (squash's j-sum, the
(u,d)-sums in u_vj1, softmax's i-sum) are d-order-agnostic, so each core's W
blob simply packs the d axis rotated by 2*core, instructions identical across
cores (SPMD), and the host concatenates the per-core (B, 2, J) outputs.

Structure: s_un[b,(d,j)] = sum_{(i,u)} xt[(iu),b] * (e[i,j]*W[(iu),(d,j)])
as bf16 matmuls (K=9216 in 72 tiles, two 128-batch halves) streaming behind
chunked input DMAs (separate tiles per chunk — dep tracking is whole-tile);
squash locally; A[(iu),(d,j)] = sum_b xb[b,(iu)]*v[b,(d,j)] as fp8e4
DoubleRow matmuls (both batch halves contracted per instruction at 0.5
cyc/row); u_vj1 via ACT-drain -> W.A mult (DVE) -> one d-halving add (Pool)
-> 8 accumulating ones-block matmuls per tile (PE does the remaining d- and
u-reduction).  Each 12-tile span then immediately produces e=exp(sum uv),
the next round's e*W panel, and that round's s-matmuls — so round r+1's
s-accumulation rides inside round r's agreement phase.  fp8 is used on the
A-path only: the batch-mean washes quantization out (rel 2.6e-3 end to end).
"""

import os
import sys

import numpy as np

for _p in ("/opt/trn_rl_repo",):
    if _p not in sys.path and os.path.isdir(_p):
        sys.path.insert(0, _p)

import ml_dtypes

NCORES = 8
B, U, I = 256, 8, 1152
J, D = 10, 16
NT = (I * U) // 128          # 72 k-tiles of the (i,u) contraction
JD = J * D                   # 160 capsule cols, d-major (d,j)
DL = D // NCORES             # 2 out_size rows per core in round 3
SL = DL * J                  # 20 output cols per core
SPAN = int(os.environ.get("K2_SPAN", "12"))  # tiles per chunk/span
NSPAN = NT // SPAN
DG = 3                       # tiles per PSUM drain group
N_DVE_DRAIN = int(os.environ.get("K2_DVE_DRAIN", "1"))   # drains per span on DVE
TREE_ON_POOL = os.environ.get("K2_TREE_POOL", "0") == "1"

_CACHE = {}


def _build_module():
    import concourse.bacc as bacc
    import concourse.mybir as mybir
    import concourse.tile as tile

    f32 = mybir.dt.float32
    bf16 = mybir.dt.bfloat16
    f8 = mybir.dt.float8e4
    AF = mybir.ActivationFunctionType
    ALU = mybir.AluOpType
    AX = mybir.AxisListType
    DR = mybir.MatmulPerfMode.DoubleRow

    # Land every ACT function we use (Exp, Ln, Copy, Square) on the one table
    # that covers them all so only a single LoadActFuncSet is emitted.
    if not hasattr(bacc, "_orig_get_activation_tables"):
        bacc._orig_get_activation_tables = bacc.get_activation_tables

        def _patched_tables(arch):
            tabs = bacc._orig_get_activation_tables(arch)
            AF_ = mybir.ActivationFunctionType
            ours = {AF_.Exp, AF_.Ln, AF_.Copy, AF_.Square, AF_.Identity}
            out = {}
            for name, s in tabs.items():
                if name == "natural_log_exp_and_others":
                    out[name] = s
                else:
                    out[name] = s - ours
            return out

        bacc.get_activation_tables = _patched_tables

    nc = bacc.Bacc(
        "TRN2", target_bir_lowering=False, debug=False, num_devices=NCORES
    )

    w8_d = nc.declare_dram_parameter("w8", [128, NT * JD], f8, isOutput=False)
    x8_d = nc.declare_dram_parameter("x8", [128, NT * B], f8, isOutput=False)
    wt_d = nc.declare_dram_parameter("wt", [128, NT * JD], bf16, isOutput=False)
    xt_d = nc.declare_dram_parameter("xt", [128, NT * B], bf16, isOutput=False)
    xb_d = nc.declare_dram_parameter("xb", [128, 2 * NT * 128], f8, isOutput=False)
    bc_d = nc.declare_dram_parameter("bcin", [128, 129], bf16, isOutput=False)
    fc_d = nc.declare_dram_parameter("fcin", [128, 304], f32, isOutput=False)
    out_d = nc.declare_dram_parameter("out", [B, SL], f32, isOutput=True)

    with tile.TileContext(nc) as tc:
        with (
            tc.tile_pool(name="const", bufs=1) as cpool,
            tc.tile_pool(name="work", bufs=2) as wpool,
            tc.tile_pool(name="psum", bufs=1, space="PSUM") as ppool,
            tc.tile_pool(name="apsum", bufs=4, space="PSUM") as apool,
        ):
            # ---- inputs, one tile per DMA chunk so s1 streams behind them ----
            fc_sb = cpool.tile([128, 304], f32)
            nc.sync.dma_start(fc_sb[:, :], fc_d[:, :])
            bc_sb = cpool.tile([128, 129], bf16)
            nc.sync.dma_start(bc_sb[:, :], bc_d[:, :])
            w8_c, x8_c = [], []
            for ch in range(NSPAN):
                w8tile = cpool.tile([128, SPAN * JD], f8, tag=f"w8{ch}", name=f"w8{ch}")
                nc.sync.dma_start(
                    w8tile[:, :], w8_d[:, ch * SPAN * JD:(ch + 1) * SPAN * JD]
                )
                w8_c.append(w8tile)
                x8tile = cpool.tile([128, SPAN * B], f8, tag=f"x8{ch}", name=f"x8{ch}")
                nc.sync.dma_start(
                    x8tile[:, :], x8_d[:, ch * SPAN * B:(ch + 1) * SPAN * B]
                )
                x8_c.append(x8tile)
            HB = NT * 128
            SB = SPAN * 128
            xb_c, wt_c, xt_c = [], [], []
            for ch in range(NSPAN):
                # xb and W16 interleave per span: span ch's A-matmuls and its
                # p-multiply (W.A) unblock together, pulling the whole
                # agreement-1 chain earlier
                xbtile = cpool.tile([128, 2, SB], f8, tag=f"xb{ch}")
                nc.sync.dma_start(
                    xbtile[:, :, :].rearrange("p h n -> p (h n)"),
                    xb_d[:, ch * 2 * SB:(ch + 1) * 2 * SB],
                )
                xb_c.append(xbtile)
                wtile = cpool.tile([128, SPAN * JD], bf16, tag=f"wt{ch}")
                nc.sync.dma_start(
                    wtile[:, :], wt_d[:, ch * SPAN * JD:(ch + 1) * SPAN * JD]
                )
                wt_c.append(wtile)
            for ch in range(NSPAN):
                xtile = cpool.tile([128, SPAN * B], bf16, tag=f"xt{ch}")
                nc.sync.dma_start(
                    xtile[:, :], xt_d[:, ch * SPAN * B:(ch + 1) * SPAN * B]
                )
                xt_c.append(xtile)

            def wvt(t):           # (128, JD) W panel of k-tile t
                return wt_c[t // SPAN][:, (t % SPAN) * JD:(t % SPAN + 1) * JD]

            def xvt(t, h):        # (128, 128) xt panel, batch half h
                c = xt_c[t // SPAN]
                return c[:, :].rearrange("p (t b) -> p t b", b=B)[
                    :, t % SPAN, h * 128:(h + 1) * 128
                ]

            def xbt(t):           # (128, 2, 128) fp8 DoubleRow lhsT of k-tile t
                return xb_c[t // SPAN][:, :, (t % SPAN) * 128:(t % SPAN + 1) * 128]

            m8 = bc_sb[:, 0:128]
            ones8 = bc_sb[:, 128:129]
            ones10 = fc_sb[0:J, 0:128]
            sel10 = fc_sb[0:J, 128:128 + JD]
            sel120 = fc_sb[0:SPAN * J, 288:298]

            # per-span round state (separate tiles: dep tracking is
            # tile-granular, one shared tile would serialize the spans)
            du_c = [cpool.tile([128, SPAN, J], f32, tag=f"du{s}", name=f"du{s}") for s in range(NSPAN)]
            for s in range(NSPAN):
                nc.vector.memset(du_c[s][:, :, :], 0.0)
            e_c = [cpool.tile([128, SPAN, J], bf16, tag=f"e{s}", name=f"e{s}") for s in range(NSPAN)]
            wc2_c = [cpool.tile([128, SPAN, JD], bf16, tag=f"wc2{s}", name=f"wc2{s}") for s in range(NSPAN)]
            wc3_c = [cpool.tile([128, SPAN, SL], bf16, tag=f"wc3{s}", name=f"wc3{s}") for s in range(NSPAN)]


            def squash(s_ps, zbc_sb, width, vdt, vtag):
                # s_ps: pair of (128,width) fp32 PSUM (batch halves);
                # zbc_sb: (128,width) f32 reciprocal softmax denominator, or a
                # float for a constant 1/z.  Returns v (128, 2, width) in vdt.
                dd = width // J
                s_n = wpool.tile([128, 2, width], f32, tag=f"s_n{vtag}")
                for h in range(2):
                    if isinstance(zbc_sb, float):
                        nc.vector.tensor_scalar_mul(s_n[:, h, :], s_ps[h][:, :], zbc_sb)
                    else:
                        nc.vector.tensor_tensor(
                            s_n[:, h, :], s_ps[h][:, :], zbc_sb[:, :], ALU.mult
                        )
                sq = wpool.tile([128, 2 * width], bf16, tag=f"sq{vtag}")
                nc.scalar.square(sq[:, :], s_n[:, :, :].rearrange("p h n -> p (h n)"))
                msq = wpool.tile([128, 2 * dd], f32, tag=f"msq{vtag}")
                nc.vector.tensor_reduce(
                    msq[:, :].rearrange("p (h d) -> p h d", h=2),
                    sq[:, :].rearrange("p (h d j) -> p h d j", h=2, j=J),
                    axis=AX.X, op=ALU.add,
                )
                lnm = wpool.tile([128, 2 * dd], f32, tag=f"lnm{vtag}")
                nc.scalar.activation(lnm[:, :], msq[:, :], AF.Ln)
                rt = wpool.tile([128, 2 * dd], f32, tag=f"rt{vtag}")
                nc.scalar.activation(rt[:, :], lnm[:, :], AF.Exp, scale=0.5)
                dn = wpool.tile([128, 2 * dd], f32, tag=f"dn{vtag}")
                nc.vector.tensor_scalar_add(dn[:, :], msq[:, :], 1.0)
                rc = wpool.tile([128, 2 * dd], f32, tag=f"rc{vtag}")
                nc.vector.reciprocal(rc[:, :], dn[:, :])
                f_t = wpool.tile([128, 2 * dd], f32, tag=f"f_t{vtag}")
                nc.vector.tensor_mul(f_t[:, :], rt[:, :], rc[:, :])
                v = wpool.tile([128, 2, width], vdt, tag=f"v{vtag}")
                for h in range(2):
                    nc.vector.tensor_tensor(
                        v[:, h, :].rearrange("p (d j) -> p d j", j=J),
                        s_n[:, h, :].rearrange("p (d j) -> p d j", j=J),
                        f_t[:, h * dd:(h + 1) * dd].unsqueeze(2).broadcast_to([128, dd, J]),
                        ALU.mult,
                    )
                return v

            def zbc_from(z_ps, width, tag):
                zcol = wpool.tile([SPAN * J, 1], f32, tag=f"zcol{tag}")
                nc.vector.tensor_copy(zcol[:, :], z_ps[:, :])
                z10 = ppool.tile([J, 1], f32, tag="uv_ps", name=f"z10{tag}")
                nc.tensor.matmul(z10[:, :], sel120[:, :], zcol[:, :], start=True, stop=True)
                zinv = wpool.tile([J, 1], f32, tag=f"zinv{tag}")
                nc.vector.reciprocal(zinv[:, :], z10[:, :])
                zsel = wpool.tile([J, width], f32, tag=f"zsel{tag}")
                nc.vector.tensor_scalar_mul(zsel[:, :], sel10[:, 0:width], zinv[:, 0:1])
                zbc_ps = ppool.tile([128, width], f32, tag="uv_ps", name=f"zbc_ps{tag}")
                nc.tensor.matmul(zbc_ps[:, :], ones10[:, :], zsel[:, :], start=True, stop=True)
                zbc_sb = wpool.tile([128, width], f32, tag=f"zbc_sb{tag}")
                nc.scalar.copy(zbc_sb[:, :], zbc_ps[:, :])
                return zbc_sb

            def a_phase(v, rnd, s_next, wc_next, wn):
                # Agreement phase for round `rnd`, fused with round rnd+1's
                # s-matmuls: per 12-tile span, A-matmuls -> drain -> p=W.A ->
                # one d-halving add -> 8 accumulating m8-matmuls (remaining
                # d- and u-reduction on the PE) -> du += uv -> e=exp(du) ->
                # wc panel -> next round's s-matmuls and z-matmuls.
                z_ps = ppool.tile([SPAN * J, 1], f32, tag="z_ps", name=f"z_ps{rnd}")
                LAG = int(os.environ.get("K2_LAG", "2"))

                def emit_s_span(sp):
                    # round rnd+1's s- and z-matmuls for span sp; emitted LAG
                    # spans late so the PE never stalls on span sp's
                    # drain->p->tree->uv->e->wc chain.
                    for tt in range(SPAN):
                        t = sp * SPAN + tt
                        nc.tensor.matmul(
                            s_next[0][:, :], xvt(t, 0), wc_next[sp][:, tt, :],
                            start=(t == 0), stop=(t == NT - 1),
                        )
                        nc.tensor.matmul(
                            s_next[1][:, :], xvt(t, 1), wc_next[sp][:, tt, :],
                            start=(t == 0), stop=(t == NT - 1),
                        )

                for sp in range(NSPAN):
                    ts = slice(sp * SPAN, (sp + 1) * SPAN)
                    a_sb = wpool.tile([128, SPAN * JD], bf16, tag="a_sb")
                    for g in range(SPAN // DG):
                        a_ps = apool.tile([128, DG * JD], f32, tag="a_ps")
                        for tt in range(DG):
                            t = sp * SPAN + g * DG + tt
                            nc.tensor.matmul(
                                a_ps[:, tt * JD:(tt + 1) * JD],
                                xbt(t),
                                v[:, :, :],
                                start=True, stop=True, perf_mode=DR,
                            )
                        dst = a_sb[:, g * DG * JD:(g + 1) * DG * JD]
                        if g < N_DVE_DRAIN:
                            nc.vector.tensor_copy(dst, a_ps[:, :])
                        else:
                            nc.scalar.copy(dst, a_ps[:, :])
                    p_sb = wpool.tile([128, SPAN * JD], bf16, tag="p_sb")
                    p_in2 = wt_c[sp] if os.environ.get("K2_XA") else a_sb
                    nc.vector.tensor_tensor(
                        p_sb[:, :], wt_c[sp][:, :], p_in2[:, :], ALU.mult
                    )
                    p4 = p_sb[:, :].rearrange("p (t d j) -> p t d j", d=D, j=J)
                    r1 = wpool.tile([128, SPAN, 8, J], bf16, tag="rt1")
                    eng = nc.gpsimd if TREE_ON_POOL else nc.vector
                    eng.tensor_tensor(
                        r1[:, :, :, :], p4[:, :, 0:8, :], p4[:, :, 8:16, :], ALU.add
                    )
                    uv_ps = ppool.tile([128, SPAN * J], f32, tag="uv_ps")
                    for q in range(8):
                        # one matmul covers the whole span: the m8 block-sum
                        # is column-independent, so rhs = the q-th d-slice of
                        # r1 over all 12 tiles at once.
                        nc.tensor.matmul(
                            uv_ps[:, :].rearrange("p (t j) -> p t j", j=J),
                            m8[:, :], r1[:, :, q, :],
                            start=(q == 0), stop=(q == 7),
                        )
                    if not os.environ.get("K2_XB"):
                        nc.vector.tensor_add(
                            du_c[sp][:, :, :], du_c[sp][:, :, :],
                            uv_ps[:, :].rearrange("p (t j) -> p t j", j=J),
                        )
                    nc.scalar.activation(e_c[sp][:, :, :], du_c[sp][:, :, :], AF.Exp)
                    nc.tensor.matmul(
                        z_ps[:, :], e_c[sp][:, :, :].rearrange("p t j -> p (t j)"),
                        ones8[:, :],
                        start=(sp == 0), stop=(sp == NSPAN - 1),
                    )
                    wd = wn // J
                    nc.vector.tensor_tensor(
                        wc_next[sp][:, :, :].rearrange("p t (d j) -> p t d j", j=J),
                        wt_c[sp][:, :].rearrange("p (t d j) -> p t d j", d=D, j=J)[:, :, 0:wd, :],
                        e_c[sp][:, :, :].unsqueeze(2).broadcast_to([128, SPAN, wd, J]),
                        ALU.mult,
                    )
                    if sp >= LAG:
                        emit_s_span(sp - LAG)
                for sp in range(NSPAN - LAG, NSPAN):
                    emit_s_span(sp)
                return z_ps

            # ---- round 1: c uniform -> wc1 = W (fp8: routing-only; rel
            # err 2.57e-3 vs 2.66e-3 all-bf16) -> DoubleRow pairs two k-tiles
            # per matmul and the fp8 operands load in half the DMA time, so
            # round 1 stops pacing on the bf16 stream entirely ----
            NPAIR = NT // 2
            s_ps = (
                ppool.tile([128, JD], f32, tag="s_ps0", name="s1_ps0"),
                ppool.tile([128, JD], f32, tag="s_ps1", name="s1_ps1"),
            )
            PPC = NPAIR // NSPAN      # tile-pairs per DMA chunk
            for P in range(NPAIR):
                ch, Pl = P // PPC, P % PPC
                x8p = x8_c[ch][:, :].rearrange("p (q i b) -> p q i b", i=2, b=B)
                w8p = w8_c[ch][:, :].rearrange("p (q i n) -> p q i n", i=2, n=JD)
                for h in range(2):
                    nc.tensor.matmul(
                        s_ps[h][:, :],
                        x8p[:, Pl, :, h * 128:(h + 1) * 128],
                        w8p[:, Pl, :, :],
                        start=(P == 0), stop=(P == NPAIR - 1),
                        perf_mode=DR,
                    )
            v1 = squash(s_ps, 1.0 / I, JD, f8, "1")

            # ---- agreement 1 + round-2 s ----
            s2_ps = (
                ppool.tile([128, JD], f32, tag="s_ps0", name="s2_ps0"),
                ppool.tile([128, JD], f32, tag="s_ps1", name="s2_ps1"),
            )
            z2_ps = a_phase(v1, 1, s2_ps, wc2_c, JD)
            v2 = squash(s2_ps, zbc_from(z2_ps, JD, "2"), JD, f8, "2")

            # ---- agreement 2 + round-3 s (this core's d-slice only) ----
            s3_ps = (
                ppool.tile([128, SL], f32, tag="s_ps0", name="s3_ps0"),
                ppool.tile([128, SL], f32, tag="s_ps1", name="s3_ps1"),
            )
            z3_ps = a_phase(v2, 2, s3_ps, wc3_c, SL)
            v3 = squash(s3_ps, zbc_from(z3_ps, SL, "3"), SL, f32, "3")
            nc.sync.dma_start(
                out_d[:, :].rearrange("(h p) n -> p h n", p=128), v3[:, :, :]
            )

    nc.finalize()
    return nc


def _prep_in_maps(x, W):
    x = np.asarray(x, np.float32)
    Wm = np.asarray(W, np.float32)[0]          # (I, J, D, U)
    xt = np.ascontiguousarray(
        x.transpose(2, 1, 0).reshape(I * U, B)
    )                                          # [(i,u), b]
    xt_blob = xt.reshape(NT, 128, B).transpose(1, 0, 2).reshape(128, NT * B)
    xb = xt.T                                  # (B, I*U)
    # span-major fp8 layout: cols = (span, half, 12*128)
    xbs = xb.reshape(B, NSPAN, SPAN * 128)
    xb_blob = np.concatenate(
        [np.stack([xbs[0:128, s], xbs[128:B, s]], axis=1) for s in range(NSPAN)],
        axis=1,
    ).reshape(128, 2 * NT * 128)

    bc = np.zeros((128, 129), np.float32)
    bc[:, 0:128] = np.kron(np.eye(16, dtype=np.float32), np.ones((8, 8), np.float32)) / B
    bc[:, 128] = 0.125
    fc = np.zeros((128, 304), np.float32)
    fc[0:J, 0:128] = 1.0
    fc[0:J, 128:128 + JD] = np.tile(np.eye(J, dtype=np.float32), (1, D))
    fc[0:SPAN * J, 288:298] = np.tile(np.eye(J, dtype=np.float32), (SPAN, 1))

    in_maps = []
    for c in range(NCORES):
        dperm = [(DL * c + k) % D for k in range(D)]
        w = Wm.transpose(0, 3, 2, 1)[:, :, dperm, :]       # (I, U, D-perm, J)
        w = w.reshape(I * U, D * J)
        wt_blob = w.reshape(NT, 128, JD).transpose(1, 0, 2).reshape(128, NT * JD)
        in_maps.append({
            "w8": wt_blob.astype(ml_dtypes.float8_e4m3fn),
            "x8": xt_blob.astype(ml_dtypes.float8_e4m3fn),
            "wt": wt_blob.astype(ml_dtypes.bfloat16),
            "xt": xt_blob.astype(ml_dtypes.bfloat16),
            "xb": xb_blob.astype(ml_dtypes.float8_e4m3fn),
            "bcin": bc.astype(ml_dtypes.bfloat16),
            "fcin": fc,
        })
    return in_maps


def run(x, W, trace=False):
    from concourse.bass_utils import run_bass_kernel_spmd

    if "nc" not in _CACHE:
        _CACHE["nc"] = _build_module()
    nc = _CACHE["nc"]
    in_maps = _prep_in_maps(x, W)
    res = run_bass_kernel_spmd(
        nc, in_maps, core_ids=list(range(NCORES)), trace=trace
    )
    # core c's (B, 2, J) slice covers d = 2c, 2c+1; reference output (B, J, D, 1)
    out = np.empty((B, J, D, 1), np.float32)
    for c in range(NCORES):
        v = np.asarray(res.results[c]["out"], np.float32).reshape(B, DL, J)
        for k in range(DL):
            out[:, :, DL * c + k, 0] = v[:, k, :]
    return out, res


def kernel(x, W):
    out, _ = run(x, W, trace=False)
    return out
